# revision 60
# baseline (speedup 1.0000x reference)
"""Trainium2 Bass kernel for nn_CrowdsClassificationCModel.

Computes, for B x (C,C,R) annotator confusion tensors:
    logits = einsum('bf,fkr->bkr', x, W).reshape(B,C,C,R) + b
    M      = softmax(logits, axis=2)           # over predicted-class d
    out    = einsum('bc,bcdr->bdr', p, M)      # (B, C, R)

Sharding: pure data-parallel over B across 8 NeuronCores; W/b replicated.

Per-core dataflow (Bs = 2048 rows; k = c*512 + d*64 + r, 32 chunks of 128
partitions; batch processed in two 1024-column halves):
  - PE:  logits chunk (128k x 1024b) = W2_chunk.T @ xT  (bf16, f32 PSUM)
  - ACT: E = exp(logits + bias_k)  PSUM->SBUF bf16; the per-partition bias
         operand carries b, so ACT does nothing but the 64 exps -- it is
         the critical engine (~67us busy, >85% occupancy)
  - PE:  S_dup (128 x 1024) = sum_d E via 0/1-mask matmuls (PSUM acc)
  - DVE: sinv = reciprocal_approx_fast(S) (only non-ACT engine that may
         read PSUM); Pool: qd = p_c * sinv
  - DVE: Eq = E * qd  in place (bf16 2x mode)
  - DVE/Pool: out_g += Eq[c,g]  elementwise c-sum into bf16 SBUF
         accumulators (split across both engines to balance load)
  - DMA out bf16 (host transposes + upcasts to f32)

The drain tail is minimized by splitting the final batch-half's last
class into a 512-column part processed mid-stream (7a) and a 512-column
part processed last (7b), with half-width ops interleaved across DVE and
Pool for the late classes and per-piece output DMAs spread over the
sync/scalar/gpsimd queues.
"""

import numpy as np
import ml_dtypes

BF = ml_dtypes.bfloat16
NCORES = 8
B_FULL = 16384
BS = B_FULL // NCORES   # 2048 rows per core
F = 128
C = 8
R = 64
K = C * C * R           # 4096
NCHUNK = K // 128       # 32 k-chunks
NB = 2                  # batch halves per core
BCH = BS // NB          # 1024

_CACHE = {}


def _build_nc():
    import concourse.bass as bass
    import concourse.bacc as bacc
    import concourse.tile as tile
    from concourse import mybir
    from contextlib import ExitStack

    f32 = mybir.dt.float32
    bf16 = mybir.dt.bfloat16
    Exp = mybir.ActivationFunctionType.Exp
    MUL = mybir.AluOpType.mult
    ADD = mybir.AluOpType.add
    DIV = mybir.AluOpType.divide

    nc = bacc.Bacc()
    xT = nc.declare_dram_parameter("xT", [128, BS], bf16, isOutput=False)
    W2 = nc.declare_dram_parameter("W2", [128, K], bf16, isOutput=False)
    pT = nc.declare_dram_parameter("pT", [C, BS], bf16, isOutput=False)
    msk = nc.declare_dram_parameter("msk", [128, 128], bf16, isOutput=False)
    bia = nc.declare_dram_parameter("bia", [128, NCHUNK], f32, isOutput=False)
    # k-major output: row k' = d*64+r, col b; host transposes after gather
    out = nc.declare_dram_parameter("out", [C * R, BS], bf16, isOutput=True)

    def ap(t):
        return t.handle if hasattr(t, "handle") else t

    with ExitStack() as ctx:
        tc = ctx.enter_context(tile.TileContext(nc))
        const = ctx.enter_context(tc.tile_pool(name="const", bufs=1))
        epool = ctx.enter_context(tc.tile_pool(name="e", bufs=6))
        sm = ctx.enter_context(tc.tile_pool(name="sm", bufs=4))
        accp = ctx.enter_context(tc.tile_pool(name="acc", bufs=2))
        plg = ctx.enter_context(tc.tile_pool(name="plg", bufs=2, space="PSUM"))
        pss = ctx.enter_context(tc.tile_pool(name="pss", bufs=2, space="PSUM"))

        # const loads: order so the first chunks' dependencies land first.
        # xT halves first (first matmul needs only the bc0 half), then the
        # first quarter of W2 (chunks j=0..7), bias (first exp), p-broadcast
        # slice (c=0 divide), mask (first S matmul), then the rest.
        xTs = const.tile([128, BS], bf16)
        W2s = const.tile([128, K], bf16)
        bias = const.tile([128, NCHUNK], f32)
        msks = const.tile([128, 128], bf16)
        pbt = []
        for bc in range(NB):
            pb_bc = const.tile([128, C, BCH], bf16, tag=f"pb{bc}")
            pbt.append(pb_bc)

        # first-chunk dependencies go out in parallel on separate queues so
        # the first exp can start ~2.5us in
        nc.sync.dma_start(out=xTs[:, 0:512], in_=xT[:, 0:512])
        nc.gpsimd.dma_start(out=W2s[:, 0:512], in_=W2[:, 0:512])
        nc.scalar.dma_start(out=bias, in_=bia[:, :])
        nc.sync.dma_start(out=xTs[:, 512:BCH], in_=xT[:, 512:BCH])
        nc.gpsimd.dma_start(out=msks, in_=msk[:, :])
        nc.sync.dma_start(
            out=pbt[0][:, 0, :],
            in_=bass.AP(tensor=ap(pT), offset=0, ap=[[0, 128], [1, BCH]]),
        )
        nc.sync.dma_start(out=W2s[:, 512:1024], in_=W2[:, 512:1024])
        nc.sync.dma_start(out=xTs[:, BCH:BS], in_=xT[:, BCH:BS])
        for i in range(1, 4):
            nc.sync.dma_start(out=W2s[:, i * 1024:(i + 1) * 1024],
                              in_=W2[:, i * 1024:(i + 1) * 1024])
        for c in range(1, C):
            nc.sync.dma_start(
                out=pbt[0][:, c, :],
                in_=bass.AP(tensor=ap(pT), offset=c * BS, ap=[[0, 128], [1, BCH]]),
            )
        for c in range(C):
            nc.sync.dma_start(
                out=pbt[1][:, c, :],
                in_=bass.AP(tensor=ap(pT), offset=c * BS + BCH,
                            ap=[[0, 128], [1, BCH]]),
            )

        def sfold(etiles, cols, tag="s"):
            """Masked d-sum matmuls into PSUM for the given column ranges.

            cols: list of (lo, width) bank-aligned sub-ranges (width <= 512).
            etiles slices are relative to the class's local column space."""
            sps = pss.tile([128, BCH], f32, tag=tag)
            for dj in range(4):
                o = 0
                for lo, w in cols:
                    nc.tensor.matmul(
                        sps[:, lo:lo + w], lhsT=msks,
                        rhs=etiles[dj][:, o:o + w],
                        start=(dj == 0), stop=(dj == 3))
                    o += w
            return sps

        def emit_sdiv(bc, c, etiles):
            """S-fold + qd = p_c / S via fast reciprocal (DVE, the only
            non-ACT engine allowed to read PSUM) then multiply by p.
            A TensorTensor divide is not a legal ISA op on real hardware."""
            sps = sfold(etiles, [(0, 512), (512, 512)])
            sinv = sm.tile([128, BCH], f32, tag="sinv")
            nc.vector.reciprocal_approx_fast(out=sinv, in_=sps)
            qd = sm.tile([128, BCH], bf16, tag="qd")
            nc.gpsimd.tensor_tensor(out=qd, in0=pbt[bc][:, c, :],
                                    in1=sinv, op=MUL)
            return qd

        def emit_eqsum(bc, c, etiles, qd, accs, e0tiles, late):
            """Eq = E * qd in place, then out_g += Eq.  Steady state: Eq on
            DVE; c-sum g0 on DVE, g1..3 on Pool.  Late classes split each
            chain evenly so neither engine carries a serial backlog into
            the drain tail."""
            if late:
                # half-width Eq+add pairs interleaved across both engines:
                # short serial chains, so neither engine drags a backlog
                # into the drain
                for h in range(2):
                    hs = slice(h * 512, (h + 1) * 512)
                    for dj in range(4):
                        eng = nc.vector if (dj + h) % 2 == 0 else nc.gpsimd
                        eng.tensor_tensor(out=etiles[dj][:, hs],
                                          in0=etiles[dj][:, hs],
                                          in1=qd[:, hs], op=MUL)
                        eng.tensor_tensor(out=accs[dj][:, hs],
                                          in0=accs[dj][:, hs],
                                          in1=etiles[dj][:, hs], op=ADD)
                return
            for dj in range(4):
                # shed some Eq load from DVE (it also runs the reciprocals)
                # onto Pool, alternating classes
                eng = nc.gpsimd if (dj == 3 and c % 2 == 0) else nc.vector
                eng.tensor_tensor(out=etiles[dj], in0=etiles[dj],
                                  in1=qd, op=MUL)
            if c == 0:
                return
            for g in range(4):
                eng = nc.vector if g == 0 else nc.gpsimd
                if c == 1:
                    eng.tensor_tensor(out=accs[g], in0=e0tiles[g],
                                      in1=etiles[g], op=ADD)
                else:
                    eng.tensor_tensor(out=accs[g], in0=accs[g],
                                      in1=etiles[g], op=ADD)

        # bc1 splits its last class into a 768-wide mid-stream part (7a)
        # and a 256-wide final part (7b) so the post-exp drain chain is a
        # quarter of a class, not a whole one.
        W7A, W7B = 768, 256
        LAST = C - 1

        def emit_chunks(bc, c, lo, w, exp_split=False):
            """Logits matmuls + biased exps for class c over local cols
            [lo, lo+w) of batch half bc.  Returns the E tiles."""
            part = "" if w == BCH else ("a" if lo == 0 else "b")
            etiles = []
            for dj in range(4):
                j = c * 4 + dj
                lg = plg.tile([128, BCH], f32, tag="lg")
                lo_abs = bc * BCH + lo
                step = 256 if (exp_split and dj == 0) else 512
                for o in range(0, w, step):
                    ws = min(step, w - o)
                    nc.tensor.matmul(
                        lg[:, lo + o:lo + o + ws],
                        lhsT=W2s[:, j * 128:(j + 1) * 128],
                        rhs=xTs[:, lo_abs + o:lo_abs + o + ws],
                        start=True, stop=True)
                E = epool.tile([128, w], bf16, tag=f"e{part}{dj}")
                if exp_split and dj == 0:
                    for h in range(2):
                        nc.scalar.activation(
                            out=E[:, h * w // 2:(h + 1) * w // 2],
                            in_=lg[:, lo + h * w // 2:lo + (h + 1) * w // 2],
                            func=Exp, bias=bias[:, j:j + 1])
                else:
                    nc.scalar.activation(out=E, in_=lg[:, lo:lo + w],
                                         func=Exp, bias=bias[:, j:j + 1])
                etiles.append(E)
            return etiles

        for bc in range(NB):
            bsl = slice(bc * BCH, (bc + 1) * BCH)
            accs = []
            for g in range(4):
                acc_g = accp.tile([128, BCH], bf16, tag=f"acc{g}")
                accs.append(acc_g)

            if bc < NB - 1:
                order = [(c, 0, BCH) for c in range(C)]
            else:
                order = ([(c, 0, BCH) for c in range(2)]
                         + [(LAST, 0, W7A)]
                         + [(c, 0, BCH) for c in range(2, C - 1)]
                         + [(LAST, W7A, W7B)])
            pend = None  # (c, lo, w, etiles, qd) with Eq/c-sum pending
            e0 = None

            for item_i, (c, lo, w) in enumerate(order):
                etiles = emit_chunks(bc, c, lo, w,
                                     exp_split=(bc == 0 and item_i == 0))
                final = (bc == NB - 1 and item_i == len(order) - 1)

                if c != LAST or bc < NB - 1:
                    qd = emit_sdiv(bc, c, etiles)
                else:
                    # partial class: fold only the produced columns
                    cols = [(o, min(512, w - o)) for o in range(0, w, 512)]
                    sps = sfold(etiles, cols, tag="s")
                    sinv7 = sm.tile([128, w], f32, tag=f"si{lo}")
                    nc.vector.reciprocal_approx_fast(out=sinv7,
                                                     in_=sps[:, 0:w])
                    qd = sm.tile([128, w], bf16, tag=f"qd{lo}")
                    eng = nc.vector if final else nc.gpsimd
                    eng.tensor_tensor(out=qd,
                                      in0=pbt[bc][:, c, lo:lo + w],
                                      in1=sinv7, op=MUL)

                # previous item's Eq/c-sum, deferred until after this item's
                # S/recip so late reciprocals never queue behind older adds
                if pend is not None:
                    pc, plo, pw, pet, pqd = pend
                    if pw == BCH:
                        late = bc == NB - 1 and item_i >= len(order) - 3
                        emit_eqsum(bc, pc, pet, pqd, accs, e0, late)
                        if pc == 0:
                            e0 = pet
                        if bc == NB - 1 and pc == C - 2:
                            # cols [0, W7A) are complete once the last full
                            # class lands (7a ran earlier): ship them
                            for g in range(4):
                                qs = [nc.sync, nc.scalar, nc.gpsimd,
                                      nc.sync][g]
                                qs.dma_start(
                                    out=out[g * 128:(g + 1) * 128,
                                            bc * BCH:bc * BCH + W7A],
                                    in_=accs[g][:, 0:W7A])
                    else:
                        # partial class 7a: half-split ops over its columns
                        for dj in range(4):
                            eng = nc.vector if dj % 2 == 0 else nc.gpsimd
                            eng.tensor_tensor(out=pet[dj], in0=pet[dj],
                                              in1=pqd, op=MUL)
                        for g in range(4):
                            eng = nc.vector if g % 2 == 1 else nc.gpsimd
                            eng.tensor_tensor(
                                out=accs[g][:, plo:plo + pw],
                                in0=accs[g][:, plo:plo + pw],
                                in1=pet[g], op=ADD)
                pend = (c, lo, w, etiles, qd)

            # drain: the final (quarter-width on bc1) item
            pc, plo, pw, pet, pqd = pend
            if bc < NB - 1:
                emit_eqsum(bc, pc, pet, pqd, accs, e0, late=True)
                for g in range(4):
                    nc.sync.dma_start(out=out[g * 128:(g + 1) * 128, bsl],
                                      in_=accs[g])
            else:
                qs = [nc.sync, nc.scalar, nc.gpsimd, nc.sync]
                for dj in range(4):
                    # quarter-width ops are cheap: keep them off Pool, whose
                    # queue may still hold late-class adds
                    eng = nc.vector if dj != 3 else nc.gpsimd
                    eng.tensor_tensor(out=pet[dj], in0=pet[dj], in1=pqd,
                                      op=MUL)
                    eng.tensor_tensor(out=accs[dj][:, plo:plo + pw],
                                      in0=accs[dj][:, plo:plo + pw],
                                      in1=pet[dj], op=ADD)
                    qs[dj].dma_start(
                        out=out[dj * 128:(dj + 1) * 128,
                                bc * BCH + plo:bc * BCH + plo + pw],
                        in_=accs[dj][:, plo:plo + pw])
    nc.compile()
    return nc


def _host_prep(x, p, W, b):
    W2 = np.ascontiguousarray(W.reshape(F, K).astype(BF))
    biaT = np.ascontiguousarray(
        b.reshape(K).astype(np.float32).reshape(NCHUNK, 128).T)  # (128, 32)
    eye64 = ((np.arange(128)[:, None] % 64) == (np.arange(128)[None, :] % 64))
    msk = eye64.astype(BF)
    xT_all = np.ascontiguousarray(x.T.astype(BF))          # (128, B)
    pT_all = np.ascontiguousarray(p.T.astype(BF))          # (8, B)
    in_maps = []
    for ci in range(NCORES):
        sl = slice(ci * BS, (ci + 1) * BS)
        in_maps.append({
            "xT": np.ascontiguousarray(xT_all[:, sl]),
            "W2": W2,
            "pT": np.ascontiguousarray(pT_all[:, sl]),
            "msk": msk,
            "bia": biaT,
        })
    return in_maps


def kernel(x, p, W, b):
    from concourse.bass_utils import run_bass_kernel_spmd

    if "nc" not in _CACHE:
        _CACHE["nc"] = _build_nc()
    nc = _CACHE["nc"]
    in_maps = _host_prep(np.asarray(x), np.asarray(p), np.asarray(W), np.asarray(b))
    res = run_bass_kernel_spmd(nc, in_maps, list(range(NCORES)))
    outs = [np.asarray(res.results[i]["out"]) for i in range(NCORES)]  # (512, BS)
    full = np.concatenate(outs, axis=1)                    # (512, B) bf16
    full = np.ascontiguousarray(full.T).astype(np.float32)  # (B, 512)
    return full.reshape(B_FULL, C, R)


# revision 66
# speedup vs baseline: 1.0124x; 1.0124x over previous
"""Trainium2 Bass kernel for nn_CrowdsClassificationCModel.

Computes, for B x (C,C,R) annotator confusion tensors:
    logits = einsum('bf,fkr->bkr', x, W).reshape(B,C,C,R) + b
    M      = softmax(logits, axis=2)           # over predicted-class d
    out    = einsum('bc,bcdr->bdr', p, M)      # (B, C, R)

Sharding: pure data-parallel over B across 8 NeuronCores; W/b replicated.

Per-core dataflow (Bs = 2048 rows; k = c*512 + d*64 + r, 32 chunks of 128
partitions; batch processed in two 1024-column halves):
  - PE:  logits chunk (128k x 1024b) = W2_chunk.T @ xT  (bf16, f32 PSUM)
  - ACT: E = exp(logits + bias_k)  PSUM->SBUF bf16; the per-partition bias
         operand carries b, so ACT does nothing but the 64 exps -- it is
         the critical engine (~67us busy, >85% occupancy)
  - PE:  S_dup (128 x 1024) = sum_d E via 0/1-mask matmuls (PSUM acc)
  - DVE: sinv = reciprocal_approx_fast(S) (only non-ACT engine that may
         read PSUM); Pool: qd = p_c * sinv
  - DVE: Eq = E * qd  in place (bf16 2x mode)
  - DVE/Pool: out_g += Eq[c,g]  elementwise c-sum into bf16 SBUF
         accumulators (split across both engines to balance load)
  - DMA out bf16 (host transposes + upcasts to f32)

The drain tail is minimized by splitting the final batch-half's last
class into a 512-column part processed mid-stream (7a) and a 512-column
part processed last (7b), with half-width ops interleaved across DVE and
Pool for the late classes and per-piece output DMAs spread over the
sync/scalar/gpsimd queues.
"""

import numpy as np
import ml_dtypes

BF = ml_dtypes.bfloat16
NCORES = 8
B_FULL = 16384
BS = B_FULL // NCORES   # 2048 rows per core
F = 128
C = 8
R = 64
K = C * C * R           # 4096
NCHUNK = K // 128       # 32 k-chunks
NB = 2                  # batch halves per core
BCH = BS // NB          # 1024

_CACHE = {}


def _build_nc():
    import concourse.bass as bass
    import concourse.bacc as bacc
    import concourse.tile as tile
    from concourse import mybir
    from contextlib import ExitStack

    f32 = mybir.dt.float32
    bf16 = mybir.dt.bfloat16
    Exp = mybir.ActivationFunctionType.Exp
    MUL = mybir.AluOpType.mult
    ADD = mybir.AluOpType.add
    DIV = mybir.AluOpType.divide

    nc = bacc.Bacc()
    xT = nc.declare_dram_parameter("xT", [128, BS], bf16, isOutput=False)
    W2 = nc.declare_dram_parameter("W2", [128, K], bf16, isOutput=False)
    pT = nc.declare_dram_parameter("pT", [C, BS], bf16, isOutput=False)
    msk = nc.declare_dram_parameter("msk", [128, 128], bf16, isOutput=False)
    bia = nc.declare_dram_parameter("bia", [128, NCHUNK], f32, isOutput=False)
    # k-major output: row k' = d*64+r, col b; host transposes after gather
    out = nc.declare_dram_parameter("out", [C * R, BS], bf16, isOutput=True)

    def ap(t):
        return t.handle if hasattr(t, "handle") else t

    with ExitStack() as ctx:
        tc = ctx.enter_context(tile.TileContext(nc))
        const = ctx.enter_context(tc.tile_pool(name="const", bufs=1))
        epool = ctx.enter_context(tc.tile_pool(name="e", bufs=6))
        sm = ctx.enter_context(tc.tile_pool(name="sm", bufs=4))
        accp = ctx.enter_context(tc.tile_pool(name="acc", bufs=2))
        plg = ctx.enter_context(tc.tile_pool(name="plg", bufs=2, space="PSUM"))
        pss = ctx.enter_context(tc.tile_pool(name="pss", bufs=2, space="PSUM"))

        # const loads: order so the first chunks' dependencies land first.
        # xT halves first (first matmul needs only the bc0 half), then the
        # first quarter of W2 (chunks j=0..7), bias (first exp), p-broadcast
        # slice (c=0 divide), mask (first S matmul), then the rest.
        xTs = const.tile([128, BS], bf16)
        W2s = const.tile([128, K], bf16)
        bias = const.tile([128, NCHUNK], f32)
        msks = const.tile([128, 128], bf16)
        pbt = []
        for bc in range(NB):
            pb_bc = const.tile([128, C, BCH], bf16, tag=f"pb{bc}")
            pbt.append(pb_bc)

        # first-chunk dependencies go out in parallel on separate queues so
        # the first exp can start ~2.5us in
        nc.sync.dma_start(out=xTs[:, 0:512], in_=xT[:, 0:512])
        nc.gpsimd.dma_start(out=W2s[:, 0:512], in_=W2[:, 0:512])
        nc.scalar.dma_start(out=bias, in_=bia[:, :])
        nc.sync.dma_start(out=xTs[:, 512:BCH], in_=xT[:, 512:BCH])
        nc.gpsimd.dma_start(out=msks, in_=msk[:, :])
        nc.sync.dma_start(
            out=pbt[0][:, 0, :],
            in_=bass.AP(tensor=ap(pT), offset=0, ap=[[0, 128], [1, BCH]]),
        )
        nc.sync.dma_start(out=W2s[:, 512:1024], in_=W2[:, 512:1024])
        nc.sync.dma_start(out=xTs[:, BCH:BS], in_=xT[:, BCH:BS])
        for i in range(1, 4):
            nc.sync.dma_start(out=W2s[:, i * 1024:(i + 1) * 1024],
                              in_=W2[:, i * 1024:(i + 1) * 1024])
        for c in range(1, C):
            nc.sync.dma_start(
                out=pbt[0][:, c, :],
                in_=bass.AP(tensor=ap(pT), offset=c * BS, ap=[[0, 128], [1, BCH]]),
            )
        for c in range(C):
            nc.sync.dma_start(
                out=pbt[1][:, c, :],
                in_=bass.AP(tensor=ap(pT), offset=c * BS + BCH,
                            ap=[[0, 128], [1, BCH]]),
            )

        def sfold(etiles, cols, tag="s"):
            """Masked d-sum matmuls into PSUM for the given column ranges.

            cols: list of (lo, width) bank-aligned sub-ranges (width <= 512).
            etiles slices are relative to the class's local column space."""
            sps = pss.tile([128, BCH], f32, tag=tag)
            for dj in range(4):
                o = 0
                for lo, w in cols:
                    nc.tensor.matmul(
                        sps[:, lo:lo + w], lhsT=msks,
                        rhs=etiles[dj][:, o:o + w],
                        start=(dj == 0), stop=(dj == 3))
                    o += w
            return sps

        def emit_sdiv(bc, c, etiles):
            """S-fold + qd = p_c / S via fast reciprocal (DVE, the only
            non-ACT engine allowed to read PSUM) then multiply by p.
            A TensorTensor divide is not a legal ISA op on real hardware."""
            sps = sfold(etiles, [(0, 512), (512, 512)])
            sinv = sm.tile([128, BCH], f32, tag="sinv")
            nc.vector.reciprocal_approx_fast(out=sinv, in_=sps)
            qd = sm.tile([128, BCH], bf16, tag="qd")
            nc.gpsimd.tensor_tensor(out=qd, in0=pbt[bc][:, c, :],
                                    in1=sinv, op=MUL)
            return qd

        def emit_eqsum(bc, c, etiles, qd, accs, e0tiles, late):
            """Eq = E * qd in place, then out_g += Eq.  Steady state: Eq on
            DVE; c-sum g0 on DVE, g1..3 on Pool.  Late classes split each
            chain evenly so neither engine carries a serial backlog into
            the drain tail."""
            if late:
                # half-width Eq+add pairs interleaved across both engines:
                # short serial chains, so neither engine drags a backlog
                # into the drain.  The very last full class leans on DVE.
                dve_pairs = {(0, 0), (0, 1), (1, 1), (2, 0), (3, 1)} \
                    if late == 2 else \
                    {(dj, h) for dj in range(4) for h in range(2)
                     if (dj + h) % 2 == 0}
                for h in range(2):
                    hs = slice(h * 512, (h + 1) * 512)
                    for dj in range(4):
                        eng = nc.vector if (dj, h) in dve_pairs else nc.gpsimd
                        eng.tensor_tensor(out=etiles[dj][:, hs],
                                          in0=etiles[dj][:, hs],
                                          in1=qd[:, hs], op=MUL)
                        eng.tensor_tensor(out=accs[dj][:, hs],
                                          in0=accs[dj][:, hs],
                                          in1=etiles[dj][:, hs], op=ADD)
                return
            for dj in range(4):
                # shed some Eq load from DVE (it also runs the reciprocals)
                # onto Pool, alternating classes
                eng = nc.gpsimd if (dj == 3 and c % 2 == 0) else nc.vector
                eng.tensor_tensor(out=etiles[dj], in0=etiles[dj],
                                  in1=qd, op=MUL)
            if c == 0:
                return
            for g in range(4):
                eng = nc.vector if g == 0 else nc.gpsimd
                if c == 1:
                    eng.tensor_tensor(out=accs[g], in0=e0tiles[g],
                                      in1=etiles[g], op=ADD)
                else:
                    eng.tensor_tensor(out=accs[g], in0=accs[g],
                                      in1=etiles[g], op=ADD)

        # bc1 splits its last class into a 768-wide mid-stream part (7a)
        # and a 256-wide final part (7b) so the post-exp drain chain is a
        # quarter of a class, not a whole one.
        W7A, W7B = 768, 256
        LAST = C - 1

        def emit_chunks(bc, c, lo, w, exp_split=False):
            """Logits matmuls + biased exps for class c over local cols
            [lo, lo+w) of batch half bc.  Returns the E tiles."""
            part = "" if w == BCH else ("a" if lo == 0 else "b")
            etiles = []
            for dj in range(4):
                j = c * 4 + dj
                lg = plg.tile([128, BCH], f32, tag="lg")
                lo_abs = bc * BCH + lo
                step = 256 if (exp_split and dj == 0) else 512
                for o in range(0, w, step):
                    ws = min(step, w - o)
                    nc.tensor.matmul(
                        lg[:, lo + o:lo + o + ws],
                        lhsT=W2s[:, j * 128:(j + 1) * 128],
                        rhs=xTs[:, lo_abs + o:lo_abs + o + ws],
                        start=True, stop=True)
                E = epool.tile([128, w], bf16, tag=f"e{part}{dj}")
                if exp_split and dj == 0:
                    for h in range(2):
                        nc.scalar.activation(
                            out=E[:, h * w // 2:(h + 1) * w // 2],
                            in_=lg[:, lo + h * w // 2:lo + (h + 1) * w // 2],
                            func=Exp, bias=bias[:, j:j + 1])
                else:
                    nc.scalar.activation(out=E, in_=lg[:, lo:lo + w],
                                         func=Exp, bias=bias[:, j:j + 1])
                etiles.append(E)
            return etiles

        for bc in range(NB):
            bsl = slice(bc * BCH, (bc + 1) * BCH)
            accs = []
            for g in range(4):
                acc_g = accp.tile([128, BCH], bf16, tag=f"acc{g}")
                accs.append(acc_g)

            if bc < NB - 1:
                order = [(c, 0, BCH) for c in range(C)]
            else:
                order = ([(c, 0, BCH) for c in range(2)]
                         + [(LAST, 0, W7A)]
                         + [(c, 0, BCH) for c in range(2, C - 1)]
                         + [(LAST, W7A, W7B)])
            pend = None  # (c, lo, w, etiles, qd) with Eq/c-sum pending
            e0 = None

            for item_i, (c, lo, w) in enumerate(order):
                etiles = emit_chunks(bc, c, lo, w,
                                     exp_split=(bc == 0 and item_i == 0))
                final = (bc == NB - 1 and item_i == len(order) - 1)

                if c != LAST or bc < NB - 1:
                    qd = emit_sdiv(bc, c, etiles)
                else:
                    # partial class: fold only the produced columns
                    cols = [(o, min(512, w - o)) for o in range(0, w, 512)]
                    sps = sfold(etiles, cols, tag="s")
                    sinv7 = sm.tile([128, w], f32, tag=f"si{lo}")
                    nc.vector.reciprocal_approx_fast(out=sinv7,
                                                     in_=sps[:, 0:w])
                    qd = sm.tile([128, w], bf16, tag=f"qd{lo}")
                    eng = nc.vector if final else nc.gpsimd
                    eng.tensor_tensor(out=qd,
                                      in0=pbt[bc][:, c, lo:lo + w],
                                      in1=sinv7, op=MUL)

                # previous item's Eq/c-sum, deferred until after this item's
                # S/recip so late reciprocals never queue behind older adds
                if pend is not None:
                    pc, plo, pw, pet, pqd = pend
                    if pw == BCH:
                        if bc == NB - 1 and item_i >= len(order) - 3:
                            late = 2 if pc == C - 2 else 1
                        else:
                            late = 0
                        emit_eqsum(bc, pc, pet, pqd, accs, e0, late)
                        if pc == 0:
                            e0 = pet
                        if bc == NB - 1 and pc == C - 2:
                            # cols [0, W7A) are complete once the last full
                            # class lands (7a ran earlier): ship them
                            for g in range(4):
                                qs = [nc.sync, nc.scalar, nc.sync,
                                      nc.scalar][g]
                                qs.dma_start(
                                    out=out[g * 128:(g + 1) * 128,
                                            bc * BCH:bc * BCH + W7A],
                                    in_=accs[g][:, 0:W7A])
                    else:
                        # partial class 7a: half-split ops over its columns
                        for dj in range(4):
                            eng = nc.vector if dj % 2 == 0 else nc.gpsimd
                            eng.tensor_tensor(out=pet[dj], in0=pet[dj],
                                              in1=pqd, op=MUL)
                        for g in range(4):
                            eng = nc.vector if g % 2 == 1 else nc.gpsimd
                            eng.tensor_tensor(
                                out=accs[g][:, plo:plo + pw],
                                in0=accs[g][:, plo:plo + pw],
                                in1=pet[g], op=ADD)
                pend = (c, lo, w, etiles, qd)

            # drain: the final (quarter-width on bc1) item
            pc, plo, pw, pet, pqd = pend
            if bc < NB - 1:
                emit_eqsum(bc, pc, pet, pqd, accs, e0, late=1)
                for g in range(4):
                    nc.sync.dma_start(out=out[g * 128:(g + 1) * 128, bsl],
                                      in_=accs[g])
            else:
                qs = [nc.sync, nc.scalar, nc.gpsimd, nc.scalar]
                for dj in range(4):
                    eng = nc.vector if dj != 3 else nc.gpsimd
                    eng.tensor_tensor(out=pet[dj], in0=pet[dj], in1=pqd,
                                      op=MUL)
                    eng.tensor_tensor(out=accs[dj][:, plo:plo + pw],
                                      in0=accs[dj][:, plo:plo + pw],
                                      in1=pet[dj], op=ADD)
                    qs[dj].dma_start(
                        out=out[dj * 128:(dj + 1) * 128,
                                bc * BCH + plo:bc * BCH + plo + pw],
                        in_=accs[dj][:, plo:plo + pw])
    nc.compile()
    return nc


def _host_prep(x, p, W, b):
    W2 = np.ascontiguousarray(W.reshape(F, K).astype(BF))
    biaT = np.ascontiguousarray(
        b.reshape(K).astype(np.float32).reshape(NCHUNK, 128).T)  # (128, 32)
    eye64 = ((np.arange(128)[:, None] % 64) == (np.arange(128)[None, :] % 64))
    msk = eye64.astype(BF)
    xT_all = np.ascontiguousarray(x.T.astype(BF))          # (128, B)
    pT_all = np.ascontiguousarray(p.T.astype(BF))          # (8, B)
    in_maps = []
    for ci in range(NCORES):
        sl = slice(ci * BS, (ci + 1) * BS)
        in_maps.append({
            "xT": np.ascontiguousarray(xT_all[:, sl]),
            "W2": W2,
            "pT": np.ascontiguousarray(pT_all[:, sl]),
            "msk": msk,
            "bia": biaT,
        })
    return in_maps


def kernel(x, p, W, b):
    from concourse.bass_utils import run_bass_kernel_spmd

    if "nc" not in _CACHE:
        _CACHE["nc"] = _build_nc()
    nc = _CACHE["nc"]
    in_maps = _host_prep(np.asarray(x), np.asarray(p), np.asarray(W), np.asarray(b))
    res = run_bass_kernel_spmd(nc, in_maps, list(range(NCORES)))
    outs = [np.asarray(res.results[i]["out"]) for i in range(NCORES)]  # (512, BS)
    full = np.concatenate(outs, axis=1)                    # (512, B) bf16
    full = np.ascontiguousarray(full.T).astype(np.float32)  # (B, 512)
    return full.reshape(B_FULL, C, R)
